# revision 2
# baseline (speedup 1.0000x reference)
"""Trainium2 Bass kernel for a 12-head attention block.

Problem (hardcoded): x [16, 1024, 768] f32, w_qkv [2304, 768], w_proj
[768, 768], b_proj [768].  out = proj(softmax(q k^T / sqrt(64)) v).

Sharding: pure data parallel over batch - 16 batches / 8 cores = 2
batches per core, no collectives.  All layout transposes happen on the
host: each core receives x^T slabs and produces out^T slabs.

v2 design (vs the fp32r baseline):
  * all matmul operands in bf16 (PSUM accumulation stays fp32): enables
    fast-weight-load and halves SBUF/DMA traffic; rel-err budget is 2e-2.
  * QK^T processes HEAD PAIRS with PE row tiling: even head at array
    rows 0-63 (tile_position (0,0)), odd head at rows 64-127 ((64,0)).
    The two K=64 matmuls run concurrently -> 2x effective throughput.
    qt/kt are laid out head-major so the partition ranges line up.
  * exp on ACT covers a head pair per instruction: S psum tile is
    [128, 1024] = [j-chunk, (even 512 | odd 512)] spanning two banks,
    amortizing the ~352-cycle ACT instruction overhead.
  * attention is ACT-bound; qkv projection of batch b+1 and output
    projection of batch b-1 are interleaved between attention blocks to
    fill the PE idle time.
  * V stays ones-augmented ([128, 65] stationary) so the softmax
    denominator falls out of the PV matmul as row 64.
"""

import numpy as np
from contextlib import ExitStack

import concourse.bass as bass
import concourse.mybir as mybir
import concourse.tile as tile
from concourse import bacc
from concourse import bass_utils

F32 = mybir.dt.float32
BF16 = mybir.dt.bfloat16
EXP = mybir.ActivationFunctionType.Exp

B, N, C = 16, 1024, 768
H, D = 12, 64
E = 3 * C
NCORES = 8
BL = B // NCORES          # batches per core
T = BL * N                # tokens per core
KC = C // 128             # feature chunks of 128
JC = N // 128             # token chunks of 128
NP = H // 2               # head pairs
SCALE = float(D) ** -0.5

_CACHE = {}


def _mm(nc, out, lhsT, rhs, **kw):
    nc.tensor.matmul(out, lhsT=lhsT, rhs=rhs, **kw)


def _build(ctx, tc):
    nc = tc.nc
    dram = ctx.enter_context(tc.tile_pool(name="dram", bufs=1, space="DRAM"))
    # x^T blocked: [kc, b, 128, N] so each per-batch chunk is one contiguous slab
    xT_d = dram.tile([KC, BL, 128, N], BF16, kind="ExternalInput", name="xTb", uniquify=False)
    # w_qkv^T as per-kc slabs [kc, 128, 2304]
    wqkv_d = dram.tile([KC, 128, E], BF16, kind="ExternalInput", name="wqkvb", uniquify=False)
    # w_proj^T per-kc slabs [kc, 128, 768]
    wproj_d = dram.tile([KC, 128, C], BF16, kind="ExternalInput", name="wprojb", uniquify=False)
    bproj_d = dram.tile([C, 1], F32, kind="ExternalInput", name="bproj", uniquify=False)
    # out^T blocked: [oc, b, 128, N]
    outT_d = dram.tile([KC, BL, 128, N], F32, kind="ExternalOutput", name="outTb", uniquify=False)

    consts = ctx.enter_context(tc.tile_pool(name="consts", bufs=1))
    wp_pool = ctx.enter_context(tc.tile_pool(name="wproj", bufs=KC))
    wqk_pool = ctx.enter_context(tc.tile_pool(name="wqk", bufs=KC))
    wv_pool = ctx.enter_context(tc.tile_pool(name="wv", bufs=KC))
    xo_pool = ctx.enter_context(tc.tile_pool(name="xo", bufs=2 * KC))
    qk_pool = ctx.enter_context(tc.tile_pool(name="qkpool", bufs=4))
    va_pool = ctx.enter_context(tc.tile_pool(name="vpool", bufs=2 * JC))
    ot_pool = ctx.enter_context(tc.tile_pool(name="otpool", bufs=2 * KC))
    pt_pool = ctx.enter_context(tc.tile_pool(name="ppool", bufs=3))
    sm_pool = ctx.enter_context(tc.tile_pool(name="small", bufs=6))
    lb_pool = ctx.enter_context(tc.tile_pool(name="lbpool", bufs=4))
    # PSUM: 8 banks total.  s-tiles 2x[128,1024] = 4 banks, O 2x[65,512]
    # = 2 banks, general (qkv/proj) 2x[128,512] = 2 banks.
    ps_s = ctx.enter_context(tc.tile_pool(name="pss", bufs=2, space="PSUM"))
    ps_o = ctx.enter_context(tc.tile_pool(name="pso", bufs=2, space="PSUM"))
    ps_gp = ctx.enter_context(tc.tile_pool(name="psgp", bufs=2, space="PSUM"))

    vones = consts.tile([128, H, 1], BF16)
    nc.vector.memset(vones, 1.0)
    bias_sb = consts.tile([128, KC], F32)
    nc.sync.dma_start(
        out=bias_sb, in_=bproj_d[:, 0].rearrange("(k p) -> p k", p=128)
    )

    # batch-0 x first: the very first matmuls wait on these, so their DMAs
    # must be at the head of the queues, ahead of the bulk weight preload.
    xt = {}
    for kc in range(KC):
        xtc = xo_pool.tile([128, N], BF16, name=f"xt0_{kc}", tag="xo")
        nc.sync.dma_start(out=xtc, in_=xT_d[kc, 0])
        xt[(0, kc)] = xtc
    # q/k weights resident: per-kc [128, 12, 128] tile, ONE wide DMA each
    wqk_t = []
    for kc in range(KC):
        t = wqk_pool.tile([128, 2 * KC, 128], BF16, name=f"wqk{kc}", tag="wqk")
        nc.sync.dma_start(out=t.rearrange("p a b -> p (a b)"), in_=wqkv_d[kc, :, 0:2 * C])
        wqk_t.append(t)
    wqk = {(j, kc): wqk_t[kc][:, j, :] for j in range(2 * KC) for kc in range(KC)}
    # v weights per-kc [128, 6, 128]
    wv = []
    for kc in range(KC):
        wvt = wv_pool.tile([128, KC, 128], BF16, name=f"wv{kc}", tag="wv")
        nc.sync.dma_start(out=wvt.rearrange("p a b -> p (a b)"), in_=wqkv_d[kc, :, 2 * C:3 * C])
        wv.append(wvt)
    wp = {}

    qt = {}
    kt = {}
    va = {}
    ot = {}

    # ---------- thunk generators (emitted lazily, interleaved) ----------

    def xdma_thunk(b):
        def go():
            for kc in range(KC):
                xtc = xo_pool.tile([128, N], BF16, name=f"xt{b}_{kc}", tag="xo")
                nc.sync.dma_start(out=xtc, in_=xT_d[kc, b])
                xt[(b, kc)] = xtc
        return go

    def wp_thunk():
        def go():
            for kc in range(KC):
                t = wp_pool.tile([128, KC, 128], BF16, name=f"wpk{kc}", tag="wp")
                nc.sync.dma_start(out=t.rearrange("p a b -> p (a b)"), in_=wproj_d[kc])
                for oc in range(KC):
                    wp[(kc, oc)] = t[:, oc, :]
        return go

    def qk_alloc(b):
        qt[b] = qk_pool.tile([128, KC, N], BF16, name=f"qt{b}", tag="qk")
        kt[b] = qk_pool.tile([128, KC, N], BF16, name=f"kt{b}", tag="qk")

    def qk_thunk(b, which, mt):
        # one [128, N] slab of q^T or k^T (head pair mt), two psum halves
        def go():
            dest = qt[b] if which == 0 else kt[b]
            for hf in range(2):
                ps = ps_gp.tile([128, 512], F32, name=f"psqk{b}_{which}_{mt}_{hf}", tag="gp")
                for kc in range(KC):
                    w = wqk[(which * KC + mt, kc)]
                    _mm(nc, ps, w, xt[(b, kc)][:, hf * 512:(hf + 1) * 512],
                        start=(kc == 0), stop=(kc == KC - 1))
                nc.vector.tensor_copy(out=dest[:, mt, hf * 512:(hf + 1) * 512], in_=ps)
        return go

    def v_thunk(b, jc):
        # V in natural [j, d] layout for token chunk jc, ones-augmented
        def go():
            vat = va_pool.tile([128, H, D + 1], BF16, name=f"va{b}_{jc}", tag="va")
            vps0 = ps_gp.tile([128, 512], F32, name=f"vps0_{b}_{jc}", tag="gp")
            vps1 = ps_gp.tile([128, 512], F32, name=f"vps1_{b}_{jc}", tag="gp")
            for kc in range(KC):
                xs = xt[(b, kc)][:, jc * 128:(jc + 1) * 128]
                wvf = wv[kc].rearrange("p a b -> p (a b)")
                _mm(nc, vps0, xs, wvf[:, 0:512],
                    start=(kc == 0), stop=(kc == KC - 1))
                _mm(nc, vps1[:, 0:256], xs, wvf[:, 512:768],
                    start=(kc == 0), stop=(kc == KC - 1))
            nc.vector.tensor_copy(
                out=vat[:, 0:8, 0:D], in_=vps0.rearrange("p (h d) -> p h d", h=8)
            )
            nc.vector.tensor_copy(
                out=vat[:, 8:12, 0:D],
                in_=vps1[:, 0:256].rearrange("p (h d) -> p h d", h=4),
            )
            nc.vector.tensor_copy(out=vat[:, :, D:D + 1], in_=vones)
            va[(b, jc)] = vat
        return go

    def proj_thunk(b, oc, hf):
        def go():
            pps = ps_gp.tile([128, 512], F32, name=f"pps{b}_{oc}_{hf}", tag="gp")
            for kc in range(KC):
                _mm(nc, pps, wp[(kc, oc)],
                    ot[(b, kc)][:, hf * 512:(hf + 1) * 512],
                    start=(kc == 0), stop=(kc == KC - 1))
            ob = sm_pool.tile([128, 512], F32, name=f"ob{b}_{oc}_{hf}", tag="sm")
            nc.vector.tensor_scalar_add(out=ob, in0=pps, scalar1=bias_sb[:, oc:oc + 1])
            nc.sync.dma_start(out=outT_d[oc, b, :, hf * 512:(hf + 1) * 512], in_=ob)
        return go

    # ---------- attention for one (batch, pair, i-half) block ----------

    def attn_block(b, p, hf):
        i0 = hf * 512
        o_e = ps_o.tile([D + 1, 512], F32, name=f"oe{b}_{p}_{hf}", tag="o")
        o_o = ps_o.tile([D + 1, 512], F32, name=f"oo{b}_{p}_{hf}", tag="o")

        def qkt(jc):
            s = ps_s.tile([128, 1024], F32, name=f"s{b}_{p}_{hf}_{jc}", tag="s")
            _mm(nc, s[:, 0:512],
                kt[b][0:D, p, jc * 128:(jc + 1) * 128],
                qt[b][0:D, p, i0:i0 + 512])
            _mm(nc, s[:, 512:1024],
                kt[b][D:128, p, jc * 128:(jc + 1) * 128],
                qt[b][D:128, p, i0:i0 + 512])
            return s

        s = qkt(0)
        for jc in range(JC):
            pt = pt_pool.tile([128, 1024], BF16, name=f"pt{b}_{p}_{hf}_{jc}", tag="pt")
            nc.scalar.activation(out=pt, in_=s, func=EXP, scale=SCALE)
            if jc + 1 < JC:
                s = qkt(jc + 1)
            _mm(nc, o_e, va[(b, jc)][:, 2 * p, :], pt[:, 0:512],
                start=(jc == 0), stop=(jc == JC - 1))
            _mm(nc, o_o, va[(b, jc)][:, 2 * p + 1, :], pt[:, 512:1024],
                start=(jc == 0), stop=(jc == JC - 1))
        # normalize: rows 0..63 divided by l (= row 64)
        for h01, o_ps in ((0, o_e), (1, o_o)):
            l_sb = sm_pool.tile([1, 512], F32, name=f"l{b}_{p}_{hf}_{h01}", tag="sm")
            nc.vector.tensor_copy(out=l_sb, in_=o_ps[D:D + 1, :])
            nc.vector.reciprocal_approx_fast(out=l_sb, in_=l_sb)
            lb = lb_pool.tile([D, 512], F32, name=f"lb{b}_{p}_{hf}_{h01}", tag="lb")
            nc.gpsimd.partition_broadcast(lb, l_sb, channels=D)
            nc.vector.tensor_mul(
                out=ot[(b, p)][h01 * D:h01 * D + D, i0:i0 + 512],
                in0=o_ps[0:D, :], in1=lb,
            )

    # ---------- program ----------

    pending = []  # interleave queue, drained between attention blocks

    for b in range(BL):
        for kc in range(KC):
            ot[(b, kc)] = ot_pool.tile([128, N], BF16, name=f"ot{b}_{kc}", tag="ot")

    # batch 0 qkv, emitted up front
    qk_alloc(0)
    for mt in range(KC):
        qk_thunk(0, 0, mt)()
        qk_thunk(0, 1, mt)()
    for jc in range(JC):
        v_thunk(0, jc)()

    # interleave queue for attention(0): x/qkv of batch 1, w_proj load
    pending.append(xdma_thunk(1))
    pending.append(wp_thunk())
    qk_alloc(1)
    for mt in range(KC):
        pending.append(qk_thunk(1, 0, mt))
        pending.append(qk_thunk(1, 1, mt))
    for jc in range(JC):
        pending.append(v_thunk(1, jc))
    n0 = len(pending)

    blocks = [(p, hf) for p in range(NP) for hf in range(2)]
    for bi, (p, hf) in enumerate(blocks):
        attn_block(0, p, hf)
        take = (n0 * (bi + 1)) // len(blocks) - (n0 * bi) // len(blocks)
        for _ in range(take):
            pending.pop(0)()

    # interleave queue for attention(1): proj of batch 0
    for oc in range(KC):
        for hf in range(2):
            pending.append(proj_thunk(0, oc, hf))
    n1 = len(pending)
    for bi, (p, hf) in enumerate(blocks):
        attn_block(1, p, hf)
        take = (n1 * (bi + 1)) // len(blocks) - (n1 * bi) // len(blocks)
        for _ in range(take):
            pending.pop(0)()

    # tail: proj of batch 1
    for oc in range(KC):
        for hf in range(2):
            proj_thunk(1, oc, hf)()


def get_nc():
    if "nc" not in _CACHE:
        nc = bacc.Bacc(None, target_bir_lowering=False, debug=False)
        with tile.TileContext(nc) as tc:
            with ExitStack() as ctx:
                _build(ctx, tc)
        nc.compile()
        _CACHE["nc"] = nc
    return _CACHE["nc"]


def _to_bf16(a):
    import ml_dtypes
    return np.asarray(a, dtype=np.float32).astype(ml_dtypes.bfloat16)


def make_in_maps(x, w_qkv, w_proj, b_proj):
    x = np.asarray(x, dtype=np.float32)
    w_qkv = np.asarray(w_qkv, dtype=np.float32)
    w_proj = np.asarray(w_proj, dtype=np.float32)
    # w_qkv^T [c, e] -> per-kc slabs [kc, 128, 2304]
    wqkvb = _to_bf16(np.ascontiguousarray(w_qkv.T.reshape(KC, 128, E)))
    # w_proj^T [c, o] -> per-kc slabs [kc, 128, 768]
    wprojb = _to_bf16(np.ascontiguousarray(w_proj.T.reshape(KC, 128, C)))
    bp = np.ascontiguousarray(b_proj.astype(np.float32).reshape(C, 1))
    in_maps = []
    for c in range(NCORES):
        # x^T [c, t] -> blocks [kc, b, 128, N]
        xT = x[c * BL:(c + 1) * BL].reshape(T, C).T  # [768, 2048]
        xb = _to_bf16(np.ascontiguousarray(
            xT.reshape(KC, 128, BL, N).transpose(0, 2, 1, 3)
        ))
        in_maps.append({"xTb": xb, "wqkvb": wqkvb, "wprojb": wprojb, "bproj": bp})
    return in_maps


def assemble_out(results):
    outs = []
    for c in range(NCORES):
        ob = results[c]["outTb"]  # [oc, b, 128, N]
        oT = ob.transpose(0, 2, 1, 3).reshape(C, T)
        outs.append(np.ascontiguousarray(oT.T).reshape(BL, N, C))
    return np.concatenate(outs, axis=0).astype(np.float32)


def kernel(x, w_qkv, w_proj, b_proj):
    nc = get_nc()
    in_maps = make_in_maps(x, w_qkv, w_proj, b_proj)
    res = bass_utils.run_bass_kernel_spmd(nc, in_maps, core_ids=list(range(NCORES)))
    return assemble_out(res.results)


# revision 4
# speedup vs baseline: 1.0716x; 1.0716x over previous
"""Trainium2 Bass kernel for a 12-head attention block.

Problem (hardcoded): x [16, 1024, 768] f32, w_qkv [2304, 768], w_proj
[768, 768], b_proj [768].  out = proj(softmax(q k^T / sqrt(64)) v).

Sharding: pure data parallel over batch - 16 batches / 8 cores = 2
batches per core, no collectives.  All layout transposes happen on the
host: each core receives x^T slabs and produces out^T slabs.

v2 design (vs the fp32r baseline):
  * all matmul operands in bf16 (PSUM accumulation stays fp32): enables
    fast-weight-load and halves SBUF/DMA traffic; rel-err budget is 2e-2.
  * QK^T processes HEAD PAIRS with PE row tiling: even head at array
    rows 0-63 (tile_position (0,0)), odd head at rows 64-127 ((64,0)).
    The two K=64 matmuls run concurrently -> 2x effective throughput.
    qt/kt are laid out head-major so the partition ranges line up.
  * exp on ACT covers a head pair per instruction: S psum tile is
    [128, 1024] = [j-chunk, (even 512 | odd 512)] spanning two banks,
    amortizing the ~352-cycle ACT instruction overhead.
  * attention is ACT-bound; qkv projection of batch b+1 and output
    projection of batch b-1 are interleaved between attention blocks to
    fill the PE idle time.
  * V stays ones-augmented ([128, 65] stationary) so the softmax
    denominator falls out of the PV matmul as row 64.
"""

import numpy as np
from contextlib import ExitStack

import concourse.bass as bass
import concourse.mybir as mybir
import concourse.tile as tile
from concourse import bacc
from concourse import bass_utils

F32 = mybir.dt.float32
BF16 = mybir.dt.bfloat16
EXP = mybir.ActivationFunctionType.Exp

B, N, C = 16, 1024, 768
H, D = 12, 64
E = 3 * C
NCORES = 8
BL = B // NCORES          # batches per core
T = BL * N                # tokens per core
KC = C // 128             # feature chunks of 128
JC = N // 128             # token chunks of 128
NP = H // 2               # head pairs
SCALE = float(D) ** -0.5

_CACHE = {}


def _mm(nc, out, lhsT, rhs, **kw):
    nc.tensor.matmul(out, lhsT=lhsT, rhs=rhs, **kw)


def _build(ctx, tc):
    nc = tc.nc
    dram = ctx.enter_context(tc.tile_pool(name="dram", bufs=1, space="DRAM"))
    # x^T blocked: [kc, b, 128, N] so each per-batch chunk is one contiguous slab
    xT_d = dram.tile([KC, BL, 128, N], BF16, kind="ExternalInput", name="xTb", uniquify=False)
    # w_qkv^T as per-kc slabs [kc, 128, 2304]
    wqkv_d = dram.tile([KC, 128, E], BF16, kind="ExternalInput", name="wqkvb", uniquify=False)
    # w_proj^T per-kc slabs [kc, 128, 768]
    wproj_d = dram.tile([KC, 128, C], BF16, kind="ExternalInput", name="wprojb", uniquify=False)
    bproj_d = dram.tile([C, 1], F32, kind="ExternalInput", name="bproj", uniquify=False)
    # out^T blocked: [oc, b, 128, N]
    outT_d = dram.tile([KC, BL, 128, N], F32, kind="ExternalOutput", name="outTb", uniquify=False)

    consts = ctx.enter_context(tc.tile_pool(name="consts", bufs=1))
    wp_pool = ctx.enter_context(tc.tile_pool(name="wproj", bufs=KC))
    wqk_pool = ctx.enter_context(tc.tile_pool(name="wqk", bufs=KC))
    wv_pool = ctx.enter_context(tc.tile_pool(name="wv", bufs=KC))
    xo_pool = ctx.enter_context(tc.tile_pool(name="xo", bufs=2 * KC))
    qk_pool = ctx.enter_context(tc.tile_pool(name="qkpool", bufs=4))
    va_pool = ctx.enter_context(tc.tile_pool(name="vpool", bufs=2 * JC))
    ot_pool = ctx.enter_context(tc.tile_pool(name="otpool", bufs=2 * KC))
    pt_pool = ctx.enter_context(tc.tile_pool(name="ppool", bufs=3))
    sm_pool = ctx.enter_context(tc.tile_pool(name="small", bufs=6))
    lb_pool = ctx.enter_context(tc.tile_pool(name="lbpool", bufs=4))
    # PSUM: 8 banks total.  s-tiles 2x[128,1024] = 4 banks, O 2x[65,512]
    # = 2 banks, general (qkv/proj) 2x[128,512] = 2 banks.
    ps_s = ctx.enter_context(tc.tile_pool(name="pss", bufs=2, space="PSUM"))
    ps_o = ctx.enter_context(tc.tile_pool(name="pso", bufs=2, space="PSUM"))
    ps_gp = ctx.enter_context(tc.tile_pool(name="psgp", bufs=2, space="PSUM"))

    vones = consts.tile([128, H, 1], BF16)
    nc.vector.memset(vones, 1.0)
    bias_sb = consts.tile([128, KC], F32)
    nc.sync.dma_start(
        out=bias_sb, in_=bproj_d[:, 0].rearrange("(k p) -> p k", p=128)
    )

    # batch-0 x first: the very first matmuls wait on these, so their DMAs
    # must be at the head of the queues, ahead of the bulk weight preload.
    xt = {}
    for kc in range(KC):
        xtc = xo_pool.tile([128, N], BF16, name=f"xt0_{kc}", tag="xo")
        nc.sync.dma_start(out=xtc, in_=xT_d[kc, 0])
        xt[(0, kc)] = xtc
    # q/k weights resident: per-kc [128, 12, 128] tile, ONE wide DMA each
    wqk_t = []
    for kc in range(KC):
        t = wqk_pool.tile([128, 2 * KC, 128], BF16, name=f"wqk{kc}", tag="wqk")
        nc.sync.dma_start(out=t.rearrange("p a b -> p (a b)"), in_=wqkv_d[kc, :, 0:2 * C])
        wqk_t.append(t)
    wqk = {(j, kc): wqk_t[kc][:, j, :] for j in range(2 * KC) for kc in range(KC)}
    # v weights per-kc [128, 6, 128]
    wv = []
    for kc in range(KC):
        wvt = wv_pool.tile([128, KC, 128], BF16, name=f"wv{kc}", tag="wv")
        nc.sync.dma_start(out=wvt.rearrange("p a b -> p (a b)"), in_=wqkv_d[kc, :, 2 * C:3 * C])
        wv.append(wvt)
    wp = {}

    qt = {}
    kt = {}
    va = {}
    ot = {}

    # ---------- micro-op groups (emitted lazily, interleaved) ----------
    # Each group is (deadline, thunk); deadline (b, p) means the group
    # must be emitted before attention block (b, p, *) is emitted.

    FAR = (99, 99)

    def xdma_thunk(b):
        def go():
            for kc in range(KC):
                xtc = xo_pool.tile([128, N], BF16, name=f"xt{b}_{kc}", tag="xo")
                nc.sync.dma_start(out=xtc, in_=xT_d[kc, b])
                xt[(b, kc)] = xtc
        return go

    def wp_thunk():
        def go():
            for kc in range(KC):
                t = wp_pool.tile([128, KC, 128], BF16, name=f"wpk{kc}", tag="wp")
                nc.sync.dma_start(out=t.rearrange("p a b -> p (a b)"), in_=wproj_d[kc])
                for oc in range(KC):
                    wp[(kc, oc)] = t[:, oc, :]
        return go

    def qk_alloc(b):
        qt[b] = qk_pool.tile([128, KC, N], BF16, name=f"qt{b}", tag="qk")
        kt[b] = qk_pool.tile([128, KC, N], BF16, name=f"kt{b}", tag="qk")

    def qk_thunk(b, which, mt, hf):
        # one [128, 512] half-slab of q^T or k^T (head pair mt)
        def go():
            dest = qt[b] if which == 0 else kt[b]
            ps = ps_gp.tile([128, 512], F32, name=f"psqk{b}_{which}_{mt}_{hf}", tag="gp")
            for kc in range(KC):
                w = wqk[(which * KC + mt, kc)]
                _mm(nc, ps, w, xt[(b, kc)][:, hf * 512:(hf + 1) * 512],
                    start=(kc == 0), stop=(kc == KC - 1))
            nc.vector.tensor_copy(out=dest[:, mt, hf * 512:(hf + 1) * 512], in_=ps)
        return go

    def v_thunk(b, jc):
        # V in natural [j, d] layout for token chunk jc, ones-augmented
        def go():
            vat = va_pool.tile([128, H, D + 1], BF16, name=f"va{b}_{jc}", tag="va")
            vps0 = ps_gp.tile([128, 512], F32, name=f"vps0_{b}_{jc}", tag="gp")
            vps1 = ps_gp.tile([128, 512], F32, name=f"vps1_{b}_{jc}", tag="gp")
            for kc in range(KC):
                xs = xt[(b, kc)][:, jc * 128:(jc + 1) * 128]
                wvf = wv[kc].rearrange("p a b -> p (a b)")
                _mm(nc, vps0, xs, wvf[:, 0:512],
                    start=(kc == 0), stop=(kc == KC - 1))
                _mm(nc, vps1[:, 0:256], xs, wvf[:, 512:768],
                    start=(kc == 0), stop=(kc == KC - 1))
            nc.vector.tensor_copy(
                out=vat[:, 0:8, 0:D], in_=vps0.rearrange("p (h d) -> p h d", h=8)
            )
            nc.vector.tensor_copy(
                out=vat[:, 8:12, 0:D],
                in_=vps1[:, 0:256].rearrange("p (h d) -> p h d", h=4),
            )
            nc.vector.tensor_copy(out=vat[:, :, D:D + 1], in_=vones)
            va[(b, jc)] = vat
        return go

    def proj_thunk(b, oc, hf):
        def go():
            pps = ps_gp.tile([128, 512], F32, name=f"pps{b}_{oc}_{hf}", tag="gp")
            for kc in range(KC):
                _mm(nc, pps, wp[(kc, oc)],
                    ot[(b, kc)][:, hf * 512:(hf + 1) * 512],
                    start=(kc == 0), stop=(kc == KC - 1))
            ob = sm_pool.tile([128, 512], F32, name=f"ob{b}_{oc}_{hf}", tag="sm")
            nc.vector.tensor_scalar_add(out=ob, in0=pps, scalar1=bias_sb[:, oc:oc + 1])
            nc.sync.dma_start(out=outT_d[oc, b, :, hf * 512:(hf + 1) * 512], in_=ob)
        return go

    # ---------- interleave queue ----------

    ops = []          # FIFO of (deadline, thunk)
    state = {"left": 0.0, "acc": 0.0}

    def drain_deadline(key):
        while ops and ops[0][0] <= key:
            ops.pop(0)[1]()

    def drain_paced():
        # emit queued groups at a steady rate across remaining jc slots
        if state["left"] > 0:
            state["acc"] += len(ops) / state["left"]
            state["left"] -= 1.0
        while state["acc"] >= 1.0 and ops:
            state["acc"] -= 1.0
            ops.pop(0)[1]()

    # ---------- attention for one (batch, pair, i-half) block ----------

    def attn_block(b, p, hf):
        drain_deadline((b, p))
        i0 = hf * 512
        o_e = ps_o.tile([D + 1, 512], F32, name=f"oe{b}_{p}_{hf}", tag="o")
        o_o = ps_o.tile([D + 1, 512], F32, name=f"oo{b}_{p}_{hf}", tag="o")

        def qkt(jc):
            s = ps_s.tile([128, 1024], F32, name=f"s{b}_{p}_{hf}_{jc}", tag="s")
            _mm(nc, s[:, 0:512],
                kt[b][0:D, p, jc * 128:(jc + 1) * 128],
                qt[b][0:D, p, i0:i0 + 512])
            _mm(nc, s[:, 512:1024],
                kt[b][D:128, p, jc * 128:(jc + 1) * 128],
                qt[b][D:128, p, i0:i0 + 512])
            return s

        s = qkt(0)
        for jc in range(JC):
            pt = pt_pool.tile([128, 1024], BF16, name=f"pt{b}_{p}_{hf}_{jc}", tag="pt")
            nc.scalar.activation(out=pt, in_=s, func=EXP, scale=SCALE)
            if jc + 1 < JC:
                s = qkt(jc + 1)
            _mm(nc, o_e, va[(b, jc)][:, 2 * p, :], pt[:, 0:512],
                start=(jc == 0), stop=(jc == JC - 1))
            _mm(nc, o_o, va[(b, jc)][:, 2 * p + 1, :], pt[:, 512:1024],
                start=(jc == 0), stop=(jc == JC - 1))
            drain_paced()
        # normalize: rows 0..63 divided by l (= row 64)
        for h01, o_ps in ((0, o_e), (1, o_o)):
            l_sb = sm_pool.tile([1, 512], F32, name=f"l{b}_{p}_{hf}_{h01}", tag="sm")
            nc.vector.tensor_copy(out=l_sb, in_=o_ps[D:D + 1, :])
            nc.vector.reciprocal_approx_fast(out=l_sb, in_=l_sb)
            lb = lb_pool.tile([D, 512], F32, name=f"lb{b}_{p}_{hf}_{h01}", tag="lb")
            nc.gpsimd.partition_broadcast(lb, l_sb, channels=D)
            nc.vector.tensor_mul(
                out=ot[(b, p)][h01 * D:h01 * D + D, i0:i0 + 512],
                in0=o_ps[0:D, :], in1=lb,
            )

    # ---------- program ----------

    for b in range(BL):
        for kc in range(KC):
            ot[(b, kc)] = ot_pool.tile([128, N], BF16, name=f"ot{b}_{kc}", tag="ot")

    # batch-0 prologue: V tiles and q/k slab 0, so attention can start early
    qk_alloc(0)
    for jc in range(JC):
        v_thunk(0, jc)()
    for which in range(2):
        for hf in range(2):
            qk_thunk(0, which, 0, hf)()

    # queue: rest of batch-0 q/k (deadline = their pair), then batch-1
    # inputs/qkv (paced), then batch-0 proj (paced during attention(1))
    for mt in range(1, KC):
        for which in range(2):
            for hf in range(2):
                ops.append(((0, mt), qk_thunk(0, which, mt, hf)))
    ops.append(((0, KC - 1), xdma_thunk(1)))
    ops.append(((1, 0), wp_thunk()))
    qk_alloc(1)
    for jc in range(JC):
        ops.append(((1, 0), v_thunk(1, jc)))
    for mt in range(KC):
        for which in range(2):
            for hf in range(2):
                ops.append(((1, mt), qk_thunk(1, which, mt, hf)))
    for oc in range(KC):
        for hf in range(2):
            ops.append((FAR, proj_thunk(0, oc, hf)))

    state["left"] = float(2 * NP * 2 * JC)  # total jc slots
    for b in range(BL):
        for p in range(NP):
            for hf in range(2):
                attn_block(b, p, hf)

    # anything left (stragglers), then tail: proj of batch 1
    drain_deadline(FAR)
    for oc in range(KC):
        for hf in range(2):
            proj_thunk(1, oc, hf)()


def get_nc():
    if "nc" not in _CACHE:
        nc = bacc.Bacc(None, target_bir_lowering=False, debug=False)
        with tile.TileContext(nc) as tc:
            with ExitStack() as ctx:
                _build(ctx, tc)
        nc.compile()
        _CACHE["nc"] = nc
    return _CACHE["nc"]


def _to_bf16(a):
    import ml_dtypes
    return np.asarray(a, dtype=np.float32).astype(ml_dtypes.bfloat16)


def make_in_maps(x, w_qkv, w_proj, b_proj):
    x = np.asarray(x, dtype=np.float32)
    w_qkv = np.asarray(w_qkv, dtype=np.float32)
    w_proj = np.asarray(w_proj, dtype=np.float32)
    # w_qkv^T [c, e] -> per-kc slabs [kc, 128, 2304]
    wqkvb = _to_bf16(np.ascontiguousarray(w_qkv.T.reshape(KC, 128, E)))
    # w_proj^T [c, o] -> per-kc slabs [kc, 128, 768]
    wprojb = _to_bf16(np.ascontiguousarray(w_proj.T.reshape(KC, 128, C)))
    bp = np.ascontiguousarray(b_proj.astype(np.float32).reshape(C, 1))
    in_maps = []
    for c in range(NCORES):
        # x^T [c, t] -> blocks [kc, b, 128, N]
        xT = x[c * BL:(c + 1) * BL].reshape(T, C).T  # [768, 2048]
        xb = _to_bf16(np.ascontiguousarray(
            xT.reshape(KC, 128, BL, N).transpose(0, 2, 1, 3)
        ))
        in_maps.append({"xTb": xb, "wqkvb": wqkvb, "wprojb": wprojb, "bproj": bp})
    return in_maps


def assemble_out(results):
    outs = []
    for c in range(NCORES):
        ob = results[c]["outTb"]  # [oc, b, 128, N]
        oT = ob.transpose(0, 2, 1, 3).reshape(C, T)
        outs.append(np.ascontiguousarray(oT.T).reshape(BL, N, C))
    return np.concatenate(outs, axis=0).astype(np.float32)


def kernel(x, w_qkv, w_proj, b_proj):
    nc = get_nc()
    in_maps = make_in_maps(x, w_qkv, w_proj, b_proj)
    res = bass_utils.run_bass_kernel_spmd(nc, in_maps, core_ids=list(range(NCORES)))
    return assemble_out(res.results)
